# revision 34
# baseline (speedup 1.0000x reference)
"""Trainium2 Bass kernel: 3x3 conv (N=16, C_in=16, C_out=64, H=W=256, pad=1).

Strategy (8 NeuronCores, data-parallel over batch N -> 2 images/core), bf16:
  - Host pads x to [2,16,258,258] (zero ring) and casts to bf16; output is
    stored bf16 on-device and upcast to fp32 on host (max rel-err ~4e-3,
    well under the 2e-2 gate).  Halves DMA traffic vs fp32 -> the PE becomes
    the bound: 8 supersteps x 48 matmuls x 512 cols = 196,608 PE cycles
    ~ 82us at 2.4GHz (the provable floor for this conv on a 128x128 PE:
    128 output lanes via the two-strip block diagonal, ceil(144/64) = 3
    contraction passes).
  - Per 64-row superstep: two 32-row strips (A,B) stacked as (kh, strip, ci)
    im2col slabs on partitions 0-95.  kh=0 / kh=2 blocks are loaded DIRECTLY
    from HBM (one 32-partition DMA each, both strips at once — the slab
    row-pitch equals the padded row pitch so each channel is one contiguous
    descriptor); the kh=1 center block is ONE Activation-engine copy from
    the kh=2 block (Act runs nothing else, ~7.1us < 10.24us PE/superstep).
  - One matmul per kw tap (3, PSUM-accumulated) with a [96,128]
    block-diagonal bf16 weight computes both strips' 64 output channels for
    512 pixels per instruction; kw shifts are free-dim offsets into the slab.
  - PSUM tiles span two banks [128,1024]; one DVE evacuation per pair
    (8 x 1192ns = 9.5us/superstep < PE) casts fp32 -> bf16.  Stores are
    per-strip [64,1024/2048] 2-level SBUF APs (3-level SBUF-side DMA APs
    mis-generate descriptors on real HW) on the SP queue; steady slab loads
    ride the Pool SWDGE queue, throttled by the 3-buffer slab pool's WAR
    dependency so lookahead DMA never crowds the FIFO DMA engines.
  - Engine layout per superstep: PE 10.24us (bound), DMA ~8.8us, DVE 9.5us,
    Act 7.1us, Pool ~2us — gapless PE at 2.4GHz after a memset-fed warm-up
    bridges the p-state ramp (full clock only after 3us continuous busy).
  - Timeline-sim: 91,626 ns vs 135,807 ns fp32r baseline (1.48x).
"""

import sys

if "/opt/trn_rl_repo" not in sys.path:
    sys.path.insert(0, "/opt/trn_rl_repo")

import ml_dtypes
import numpy as np

import concourse.bacc as bacc
import concourse.bass as bass
import concourse.mybir as mybir
import concourse.tile as tile
from concourse.bass_utils import run_bass_kernel_spmd

N_FULL, CI, CO, H, W_SP = 16, 16, 64, 256, 256
NCORES = 8
NB = N_FULL // NCORES          # batches per core
HP, WP = H + 2, W_SP + 2       # padded image dims
SLOT = WP                      # 258: one row-slot in the slab (z x0..x255 z)
RSTRIP = 32                    # output rows per strip
SLOTS = RSTRIP + 2             # row-slots per strip slab (rows + 2 halo)
NSS = H // (2 * RSTRIP)        # supersteps per image (4)
BF16 = mybir.dt.bfloat16
F32 = mybir.dt.float32
BF16NP = ml_dtypes.bfloat16

N_WARM = 26                    # PE ramp-bridging dummy matmuls
LOOK_LOAD = 3                  # supersteps of load lookahead
FULL3 = 2                      # prologue slabs loaded with all three kh blocks
N_CHUNK = 1                    # center-copy chunks (Act engine is dedicated)

_CACHE = {}


def _build():
    nc = bacc.Bacc("TRN2", target_bir_lowering=False, debug=False)
    x_d = nc.dram_tensor("xp", [NB, CI, HP, WP], BF16, kind="ExternalInput").ap()
    w_d = nc.dram_tensor("wts", [96, 3 * 128], BF16, kind="ExternalInput").ap()
    o_d = nc.dram_tensor("out", [NB, CO, H, W_SP], BF16, kind="ExternalOutput").ap()

    xe_n = CI * HP * WP        # x_pad element strides
    xe_c = HP * WP
    xe_h = WP
    oe_n = CO * H * W_SP       # out element strides
    oe_c = H * W_SP
    oe_h = W_SP

    with tile.TileContext(nc) as tc:
        with (
            tc.tile_pool(name="wp", bufs=1) as wpool,
            tc.tile_pool(name="slab", bufs=3) as slabpool,
            tc.tile_pool(name="evac", bufs=6) as evacpool,
            tc.tile_pool(name="ps", bufs=4, space="PSUM") as pspool,
        ):
            # weights arrive pre-laid-out [96, 3*128]: one DMA on the Pool
            # SWDGE queue, keeping the serial HWDGE slots for the slab-0 head
            wsb = wpool.tile([96, 3 * 128], BF16)
            nc.gpsimd.dma_start(wsb[:], w_d)

            # PE warm-up: garbage matmuls on a memset tile (ready ~0.6us,
            # long before any DMA lands) keep the PE busy until the first
            # slab arrives, so the 3us p-state ramp to 2.4GHz completes
            # before real work starts.  The warm PSUM tile comes from the ps
            # rotation (PSUM is exactly 8 banks = 4 double-tiles); warms
            # finish before the 4th real pair needs the bank back.
            wtile = wpool.tile([96, 128], BF16, name="wtile")
            nc.vector.memset(wtile[:], 0.0)
            warm = pspool.tile([128, 1024], F32, tag="ps")
            for _ in range(N_WARM):
                nc.tensor.matmul(
                    warm[:, 0:128], wtile[:], wtile[:], start=True, stop=True
                )

            def issue_loads(i, full3, split=False):
                # slab partitions: kh*32 + strip*16 + ci.  One DMA per kh
                # block covers both strips (outer level = strip, 32 rows
                # apart in xp; slab row-pitch == padded row pitch so a
                # multi-row load is one contiguous chunk per channel).
                # full3: center block loaded straight from HBM (prologue
                # only) so no Act-queue copy gates the pipeline fill.
                # split: blocks land as three segments so the first j-groups
                # can start after ~a quarter of the bytes (slab 0 only).
                n, t = divmod(i, NSS)
                h0 = 2 * RSTRIP * t
                slab = slabpool.tile([96, SLOTS * SLOT], BF16, tag="slab")
                sf = slab[:]
                dma = nc.sync.dma_start if i < LOOK_LOAD else nc.gpsimd.dma_start
                if full3:
                    # per-kh-block loads (DMA APs are capped at 3 dims, so
                    # the three blocks cannot merge into one descriptor set);
                    # all three write the same dest slots 1..32
                    segs = ((1, 9), (9, 17), (17, 33)) if split else ((1, 33),)
                    for lo, hi in segs:
                        for kh in range(3):
                            src = bass.AP(
                                x_d.tensor,
                                n * xe_n + (h0 + kh + lo - 1) * xe_h,
                                [[RSTRIP * WP, 2], [xe_c, CI], [1, (hi - lo) * WP]],
                            )
                            dma(sf[32 * kh : 32 * kh + 32, lo * SLOT : hi * SLOT], src)
                else:
                    # kh=0 slots 1..32; kh=2 slots 0..32 (slot 0 feeds the
                    # center-block copy)
                    for p0, lo, hi, row in ((0, 1, 33, h0), (64, 0, 33, h0 + 1)):
                        src = bass.AP(
                            x_d.tensor,
                            n * xe_n + row * xe_h,
                            [[RSTRIP * WP, 2], [xe_c, CI], [1, (hi - lo) * WP]],
                        )
                        dma(sf[p0 : p0 + 32, lo * SLOT : hi * SLOT], src)
                return slab

            def emit_center_chunks(slab):
                # center slot u (1..32) = kh2 slot u-1.  The Act engine runs
                # nothing else, so a single big copy is fine.
                sf = slab[:]
                rows = RSTRIP // N_CHUNK
                for q in range(N_CHUNK):
                    nc.scalar.copy(
                        sf[32:64, (1 + rows * q) * SLOT : (1 + rows * (q + 1)) * SLOT],
                        sf[64:96, rows * q * SLOT : rows * (q + 1) * SLOT],
                    )

            def compute(i, slab):
                n, t = divmod(i, NSS)
                su = slab[:].rearrange("p (u e) -> p u e", u=SLOTS)
                for j in range(4):
                    evac = evacpool.tile([128, 4 * 512], BF16, tag="evac")
                    last_group = (i == NB * NSS - 1 and j == 3)
                    for pair in range(2):
                        # two PSUM banks (= two row-pair tiles) per DVE evac:
                        # halves the per-copy PSUM-access overhead and the
                        # DVE instruction count (8x1192ns < PE 10.24us)
                        ps = pspool.tile([128, 1024], F32, tag="ps")
                        for q in range(2):
                            b = 4 * j + 2 * pair + q
                            for kw in range(3):
                                rhs = su[:, 2 * b + 1 : 2 * b + 3, kw : kw + 256]
                                nc.tensor.matmul(
                                    ps[:, q * 512 : (q + 1) * 512],
                                    wsb[:, kw * 128 : (kw + 1) * 128],
                                    rhs,
                                    start=(kw == 0),
                                    stop=(kw == 2),
                                )
                        nc.vector.tensor_copy(
                            evac[:, pair * 1024 : (pair + 1) * 1024], ps[:]
                        )
                    # per-strip stores: SBUF-side DMA APs must stay 2-level
                    # (3-level SBUF sources mis-generate descriptors on real
                    # hardware).  The very last group stores in column halves
                    # so the final evac->store->sem drain chain is short.
                    nsplit = 2 if last_group else 1
                    cols = 2048 // nsplit
                    for q in range(nsplit):
                        for strip in range(2):
                            dst = bass.AP(
                                o_d.tensor,
                                n * oe_n + strip * RSTRIP * oe_h
                                + (2 * RSTRIP * t + 8 * j + q * (8 // nsplit)) * oe_h,
                                [[oe_c, CO], [1, cols]],
                            )
                            nc.sync.dma_start(
                                dst,
                                evac[strip * 64 : (strip + 1) * 64,
                                     q * cols : (q + 1) * cols],
                            )

            # slab pool has 3 buffers: loads for slab i+3 reuse slab i's
            # buffer, so the WAR dependency on compute(i) naturally throttles
            # lookahead DMA — without it, far-future SWDGE loads dispatch
            # immediately and crowd slab 0/1 off the FIFO DMA engines.
            steps = NB * NSS
            slabs = {}
            for s in range(min(FULL3, steps)):
                slabs[s] = issue_loads(s, full3=True, split=(s == 0))
            for s in range(FULL3, min(LOOK_LOAD, steps)):
                slabs[s] = issue_loads(s, full3=False)
            for i in range(steps):
                compute(i, slabs.pop(i))
                if i + LOOK_LOAD < steps and i + LOOK_LOAD >= FULL3:
                    slabs[i + LOOK_LOAD] = issue_loads(i + LOOK_LOAD, full3=False)
                if i + 2 < steps and i + 2 >= FULL3:
                    emit_center_chunks(slabs[i + 2])

    nc.compile()
    return nc


def _prep_weights(W: np.ndarray) -> np.ndarray:
    # lhsT[kw][kh*32 + strip*16 + ci, strip*64 + co] = W[co, ci, kh, kw],
    # flattened to [96, 3*128] (kw along the free dim) for a single DMA
    wts = np.zeros((3, 96, 128), dtype=np.float32)
    blk = np.ascontiguousarray(W.transpose(3, 2, 1, 0))  # [kw, kh, ci, co]
    for kh in range(3):
        for strip in range(2):
            wts[:, kh * 32 + strip * 16 : kh * 32 + (strip + 1) * 16,
                strip * 64 : (strip + 1) * 64] = blk[:, kh]
    return np.ascontiguousarray(wts.transpose(1, 0, 2)).reshape(96, 3 * 128)


def _prep_inputs(x: np.ndarray, W: np.ndarray) -> list:
    wts = _prep_weights(np.asarray(W, dtype=np.float32)).astype(BF16NP)
    xb = np.asarray(x, dtype=np.float32).astype(BF16NP)
    xp = np.zeros((NCORES, NB, CI, HP, WP), dtype=BF16NP)
    xp[:, :, :, 1 : H + 1, 1 : W_SP + 1] = xb.reshape(NCORES, NB, CI, H, W_SP)
    return [{"xp": xp[i], "wts": wts} for i in range(NCORES)]


def kernel(x: np.ndarray, W: np.ndarray) -> np.ndarray:
    assert x.shape == (N_FULL, CI, H, W_SP) and W.shape == (CO, CI, 3, 3)
    # BASS_TRACE without the axon NTFF hook module would crash the run path;
    # disable tracing only when the hook is genuinely unavailable.
    try:
        import antenv.axon_hooks  # noqa: F401
    except Exception:
        import os

        os.environ.setdefault("BASS_NEVER_TRACE", "1")
    if "nc" not in _CACHE:
        _CACHE["nc"] = _build()
    nc = _CACHE["nc"]

    in_maps = _prep_inputs(x, W)
    res = run_bass_kernel_spmd(nc, in_maps, list(range(NCORES)))
    out = np.concatenate(
        [res.results[i]["out"].astype(np.float32) for i in range(NCORES)], axis=0
    )
    return out


if __name__ == "__main__":
    import concourse.timeline_sim as tls

    nc = _build()
    ts = tls.TimelineSim(nc, trace=False)
    print(f"TimelineSim: {int(ts.simulate())} ns")



# revision 43
# speedup vs baseline: 1.1103x; 1.1103x over previous
"""Trainium2 Bass kernel: 3x3 conv (N=16, C_in=16, C_out=64, H=W=256, pad=1).

Strategy (8 NeuronCores, data-parallel over batch N -> 2 images/core), bf16,
dense row-pair matmul scheme:
  - Host pads x to [2,16,258,260] (zero ring + 2 spare cols) and casts to
    bf16; output is stored bf16 on-device and upcast to fp32 on host
    (max rel-err ~4e-3, well under the 2e-2 gate).
  - KEY IDEA: adjacent output rows share most of their receptive field, so
    a 4-row x 2-col x 16-ci input window (= 128 partitions, partition
    p = dc*64 + dr*16 + ci) densely feeds 128 output lanes
    (m = parity*64 + co, i.e. 64 channels x 2 adjacent rows 2*rp+parity).
    Pass A (lhsT [128,128]) covers kw in {0,1} for all kh; pass B
    (lhsT [64,128], the dc=0 partitions re-read at +1 element) covers kw=2.
    2 passes x 65,536 columns = 131,072 PE cycles ~ 54.6us/core — 1.5x less
    than the 3-pass block-diagonal scheme (the sparse-block 196,608-cycle
    "floor" assumed lanes must share identical input vectors; dense
    row-pair sharing beats it).  The kernel is then DMA-bound
    (~8.8us/superstep: 1.06MB loads + 2MB stores at 360GB/s).
  - Slab per 64-row superstep: partition (dc,dr,ci) slot rp holds xp row
    (r0+dr+2*rp) — a stride-2 row stream; dc=0 loads straight from HBM
    (4 dr-block DMAs, 258-elem descriptors), dc=1 is a one-element-shifted
    copy of dc=0 built on Act+GpSimd (the spare xp pad column makes the
    cross-slot seam element a correct zero).
  - PSUM tiles span two banks [128,1024] (= 2 f-blocks of 2 row-pairs x
    256 cols); evacs split 6 DVE / 2 Act pairs; stores are per-parity
    [64,2048] 2-level SBUF APs onto stride-2 output rows (3-level DRAM
    dst is fine; 3-level SBUF-side DMA APs mis-generate descriptors).
  - Memset-fed PE warm-up bridges the p-state ramp (2.4GHz only after
    3us of continuous PE busy).
"""

import sys

if "/opt/trn_rl_repo" not in sys.path:
    sys.path.insert(0, "/opt/trn_rl_repo")

import ml_dtypes
import numpy as np

import concourse.bacc as bacc
import concourse.bass as bass
import concourse.mybir as mybir
import concourse.tile as tile
from concourse.bass_utils import run_bass_kernel_spmd

N_FULL, CI, CO, H, W_SP = 16, 16, 64, 256, 256
NCORES = 8
NB = N_FULL // NCORES          # batches per core
HP, WP = H + 2, W_SP + 4       # padded image dims (2 spare cols on the right)
RSS = 64                       # output rows per superstep
NSS = H // RSS                 # supersteps per image (4)
NRP = RSS // 2                 # row-pairs per superstep (32)
SLOT = 258                     # elements per row-slot in the slab
FREE = NRP * SLOT              # slab free size per partition (8256)
BF16 = mybir.dt.bfloat16
F32 = mybir.dt.float32
BF16NP = ml_dtypes.bfloat16

N_WARM = 26                    # PE ramp-bridging dummy matmuls
LOOK_LOAD = 3                  # supersteps of load lookahead
ACT_COPY_SPLIT = 3700          # dc=1 copy: first part on Act, rest on GpSimd

_CACHE = {}


def _build():
    nc = bacc.Bacc("TRN2", target_bir_lowering=False, debug=False)
    x_d = nc.dram_tensor("xp", [NB, CI, HP, WP], BF16, kind="ExternalInput").ap()
    w_d = nc.dram_tensor("wts", [128, 2 * 128], BF16, kind="ExternalInput").ap()
    o_d = nc.dram_tensor("out", [NB, CO, H, W_SP], BF16, kind="ExternalOutput").ap()

    xe_n = CI * HP * WP        # x_pad element strides
    xe_c = HP * WP
    xe_h = WP
    oe_n = CO * H * W_SP       # out element strides
    oe_c = H * W_SP
    oe_h = W_SP

    with tile.TileContext(nc) as tc:
        with (
            tc.tile_pool(name="wp", bufs=1) as wpool,
            tc.tile_pool(name="slab", bufs=3) as slabpool,
            tc.tile_pool(name="evac", bufs=6) as evacpool,
            tc.tile_pool(name="ps", bufs=4, space="PSUM") as pspool,
        ):
            # weights [128, 256]: cols 0..127 = pass-A lhsT, cols 128..255
            # rows 0..63 = pass-B lhsT.  One DMA on the Pool SWDGE queue.
            wsb = wpool.tile([128, 2 * 128], BF16)
            nc.gpsimd.dma_start(wsb[:], w_d)

            # PE warm-up on a memset tile (see module docstring)
            wtile = wpool.tile([128, 128], BF16, name="wtile")
            nc.vector.memset(wtile[:], 0.0)
            warm = pspool.tile([128, 1024], F32, tag="ps")
            for _ in range(N_WARM):
                nc.tensor.matmul(
                    warm[:, 0:128], wtile[:], wtile[:], start=True, stop=True
                )

            def issue_loads(i, full8=False, split=False):
                # dc=0 half: 4 dr-block DMAs, each a stride-2-row stream of
                # 32 slots x 258 contiguous elements per ci channel.
                # slot rp of partition dc*64+dr*16+ci = xp[n,ci,r0+dr+2rp,1-dc:]
                # full8 (prologue): the dc=1 half loads straight from HBM too
                # (source offset 0), so no engine copy gates the pipeline fill.
                n, t = divmod(i, NSS)
                r0 = RSS * t
                slab = slabpool.tile([128, FREE], BF16, tag="slab")
                sf = slab[:]
                ndc = 2 if full8 else 1
                # slab 0 lands in two 16-slot halves so the first j-groups
                # start after ~half the bytes
                segs = ((0, 16), (16, NRP)) if split else ((0, NRP),)
                for lo, hi in segs:
                    for dc in range(ndc):
                        for dr in range(4):
                            src = bass.AP(
                                x_d.tensor,
                                n * xe_n + (r0 + dr + 2 * lo) * xe_h + 1 - dc,
                                [[xe_c, CI], [2 * xe_h, hi - lo], [1, SLOT]],
                            )
                            # steady loads split SP/SWDGE so neither queue's
                            # engine budget overflows
                            if full8:
                                dma = nc.sync.dma_start
                            else:
                                dma = nc.sync.dma_start if dr < 2 else nc.gpsimd.dma_start
                            dma(
                                sf[dc * 64 + dr * 16 : dc * 64 + dr * 16 + 16,
                                   lo * SLOT : hi * SLOT],
                                src,
                            )
                return slab

            def emit_dc1_copy(slab):
                # dc=1 half = dc=0 shifted one element right.  The seam
                # element (slot rp, e=0) picks up dc=0's (rp-1, e=257) =
                # xp col 259 = pad zero, which IS the correct value
                # (xp col 0 = pad) for every slot except slot 0 — fixed by
                # the 1-element memset.  Split Act / GpSimd by free range.
                sf = slab[:]
                nc.vector.memset(sf[64:128, 0:1], 0.0)
                nc.scalar.copy(
                    sf[64:128, 1 : 1 + ACT_COPY_SPLIT],
                    sf[0:64, 0:ACT_COPY_SPLIT],
                )
                nc.gpsimd.tensor_copy(
                    sf[64:128, 1 + ACT_COPY_SPLIT : FREE],
                    sf[0:64, ACT_COPY_SPLIT : FREE - 1],
                )

            def compute(i, slab):
                n, t = divmod(i, NSS)
                r0 = RSS * t
                su = slab[:].rearrange("p (u e) -> p u e", u=NRP)
                for j in range(4):
                    # j-group = 4 f-blocks = 8 row-pairs = 16 output rows
                    evac = evacpool.tile([128, 4 * 512], BF16, tag="evac")
                    for pair in range(2):
                        ps = pspool.tile([128, 1024], F32, tag="ps")
                        for q in range(2):
                            b = 4 * j + 2 * pair + q   # f-block index 0..15
                            out = ps[:, q * 512 : (q + 1) * 512]
                            # pass A: kw 0,1 — all 128 partitions, offset 0
                            nc.tensor.matmul(
                                out,
                                wsb[:, 0:128],
                                su[:, 2 * b : 2 * b + 2, 0:256],
                                start=True,
                                stop=False,
                            )
                            # pass B: kw 2 — dc=0 partitions, +1 element
                            nc.tensor.matmul(
                                out,
                                wsb[0:64, 128:256],
                                su[0:64, 2 * b : 2 * b + 2, 1:257],
                                start=False,
                                stop=True,
                            )
                        # 2 banks per evac; Act takes the second pair of
                        # each group (DVE alone would exceed the DMA bound)
                        seg = evac[:, pair * 1024 : (pair + 1) * 1024]
                        if pair == 1 and j in (1, 3):
                            nc.scalar.copy(seg, ps[:])
                        else:
                            nc.vector.tensor_copy(seg, ps[:])
                    # stores: one per parity; dst rows r0+16j+parity step 2.
                    # The last group splits by pair-half so the final store
                    # only waits on the final pair's evac.
                    last_group = (i == NB * NSS - 1 and j == 3)
                    nsp = 2 if last_group else 1
                    cols = 2048 // nsp
                    for parity in range(2):
                        for h in range(nsp):
                            dst = bass.AP(
                                o_d.tensor,
                                n * oe_n
                                + (r0 + 16 * j + 8 * h + parity) * oe_h,
                                [[oe_c, CO], [2 * oe_h, 8 // nsp], [1, W_SP]],
                            )
                            nc.sync.dma_start(
                                dst,
                                evac[parity * 64 : (parity + 1) * 64,
                                     h * cols : (h + 1) * cols],
                            )

            steps = NB * NSS
            slabs = {}
            slabs[0] = issue_loads(0, full8=True, split=True)
            slabs[1] = issue_loads(1, full8=True)
            if steps > 2:
                slabs[2] = issue_loads(2)
                emit_dc1_copy(slabs[2])
            for i in range(steps):
                compute(i, slabs.pop(i))
                if i + LOOK_LOAD < steps:
                    slabs[i + LOOK_LOAD] = issue_loads(i + LOOK_LOAD)
                if i + 2 < steps and i + 2 >= LOOK_LOAD:
                    emit_dc1_copy(slabs[i + 2])

    nc.compile()
    return nc


def _prep_weights(W: np.ndarray) -> np.ndarray:
    # pass A: wA[dc*64 + dr*16 + ci, parity*64 + co] =
    #   W[co, ci, kh=dr-parity, kw=1-dc]  (0 where dr-parity not in 0..2)
    # pass B: wB[dr*16 + ci, parity*64 + co] = W[co, ci, dr-parity, 2]
    wts = np.zeros((128, 2 * 128), dtype=np.float32)
    for dc in range(2):
        for dr in range(4):
            for parity in range(2):
                kh = dr - parity
                if 0 <= kh <= 2:
                    blk = W[:, :, kh, 1 - dc].T  # [ci, co]
                    wts[dc * 64 + dr * 16 : dc * 64 + dr * 16 + 16,
                        parity * 64 : (parity + 1) * 64] = blk
                    if dc == 0:
                        wts[dr * 16 : dr * 16 + 16,
                            128 + parity * 64 : 128 + (parity + 1) * 64] = (
                            W[:, :, kh, 2].T
                        )
    return wts


def _prep_inputs(x: np.ndarray, W: np.ndarray) -> list:
    wts = _prep_weights(np.asarray(W, dtype=np.float32)).astype(BF16NP)
    xb = np.asarray(x, dtype=np.float32).astype(BF16NP)
    xp = np.zeros((NCORES, NB, CI, HP, WP), dtype=BF16NP)
    xp[:, :, :, 1 : H + 1, 1 : W_SP + 1] = xb.reshape(NCORES, NB, CI, H, W_SP)
    return [{"xp": xp[i], "wts": wts} for i in range(NCORES)]


def kernel(x: np.ndarray, W: np.ndarray) -> np.ndarray:
    assert x.shape == (N_FULL, CI, H, W_SP) and W.shape == (CO, CI, 3, 3)
    # BASS_TRACE without the axon NTFF hook module would crash the run path;
    # disable tracing only when the hook is genuinely unavailable.
    try:
        import antenv.axon_hooks  # noqa: F401
    except Exception:
        import os

        os.environ.setdefault("BASS_NEVER_TRACE", "1")
    if "nc" not in _CACHE:
        _CACHE["nc"] = _build()
    nc = _CACHE["nc"]

    in_maps = _prep_inputs(x, W)
    res = run_bass_kernel_spmd(nc, in_maps, list(range(NCORES)))
    out = np.concatenate(
        [res.results[i]["out"].astype(np.float32) for i in range(NCORES)], axis=0
    )
    return out


if __name__ == "__main__":
    import concourse.timeline_sim as tls

    nc = _build()
    ts = tls.TimelineSim(nc, trace=False)
    print(f"TimelineSim: {int(ts.simulate())} ns")
